# revision 15
# baseline (speedup 1.0000x reference)
"""Trainium2 Bass kernel for nn_Net_66408784331557 (dense MLP with sync-BN).

Reference computation:
    h = BN_train(x; gamma_in, beta_in)            # x: [65536, 2048]
    h = relu(h @ W_in.T + b_in)                   # -> [65536, 75]
    12x: h = relu(BN_train(h; g_l, b_l) @ W_l.T + bias_l)
    out = h @ W_out.T + b_out                     # -> [65536, 1]

Strategy: data-parallel over the batch across 8 NeuronCores (8192 rows each).
BatchNorm is algebraically folded into the following Linear layer:
    BN(x) @ W.T = x @ (W * s).T + (beta - mu*s) @ W.T,  s = gamma*rsqrt(var+eps)
so each layer needs only the global per-feature (sum, sumsq) -> one tiny
AllReduce per BN, then a weight fold, then a plain matmul + fused ReLU.

Layer 1 is two passes over x (stats pass, then matmul pass); the 12 middle
layers keep h resident in SBUF in [feature, batch] layout.
"""

import sys
import os
import functools

import numpy as np

for _p in ("/opt/trn_rl_repo",):
    if _p not in sys.path:
        sys.path.insert(0, _p)

import ml_dtypes

N_CORES = 8
B = 65536
D = 2048
H = 75
L = 12
N_OUT = 1
EPS = 1e-5

BF16 = ml_dtypes.bfloat16


def build_program(n_cores=N_CORES, b_local=B // N_CORES, d=D, h=H, n_layers=L,
                  debug=False):
    """Builds the SPMD Bass/Tile program (identical on every core)."""
    import concourse.bass as bass
    import concourse.mybir as mybir
    import concourse.tile as tile
    from concourse import bacc

    f32 = mybir.dt.float32
    f32r = mybir.dt.float32r
    bf16 = mybir.dt.bfloat16
    AF = mybir.ActivationFunctionType
    ALU = mybir.AluOpType

    QD = d // 128          # feature chunks of 128
    CC = d // 512          # colsum chunks of 512
    NT = b_local // 128    # pass-1 tiles (128 rows each)
    BCN = b_local // 512   # batch chunks of 512
    B_TOT = n_cores * b_local

    nc = bacc.Bacc("TRN2", target_bir_lowering=False, debug=debug,
                   enable_asserts=True, num_devices=n_cores)

    # ---- I/O ----
    x_d = nc.dram_tensor("x", [b_local, d], f32, kind="ExternalInput").ap()
    wint_d = nc.dram_tensor("wint", [d, h], f32, kind="ExternalInput").ap()
    bin_d = nc.dram_tensor("bin", [h, 1], f32, kind="ExternalInput").ap()
    grow_d = nc.dram_tensor("grow", [1, d], f32, kind="ExternalInput").ap()
    brow_d = nc.dram_tensor("brow", [1, d], f32, kind="ExternalInput").ap()
    midwt_d = nc.dram_tensor("midwt", [n_layers, h, h], f32, kind="ExternalInput").ap()
    midg_d = nc.dram_tensor("midg", [h, n_layers], f32, kind="ExternalInput").ap()
    midbeta_d = nc.dram_tensor("midbeta", [h, n_layers], f32, kind="ExternalInput").ap()
    midbias_d = nc.dram_tensor("midbias", [h, n_layers], f32, kind="ExternalInput").ap()
    woutt_d = nc.dram_tensor("woutt", [h, N_OUT], f32, kind="ExternalInput").ap()
    bout_d = nc.dram_tensor("bout", [1, 1], f32, kind="ExternalInput").ap()
    identf_d = nc.dram_tensor("identf", [128, 128], f32, kind="ExternalInput").ap()
    identf2_d = nc.dram_tensor("identf2", [2, 2], f32, kind="ExternalInput").ap()
    onesbf_d = nc.dram_tensor("onesbf", [128, 1], bf16, kind="ExternalInput").ap()
    out_d = nc.dram_tensor("out", [b_local, N_OUT], f32, kind="ExternalOutput").ap()

    rg = [list(range(n_cores))]

    with tile.TileContext(nc) as tc:
        with tc.tile_pool(name="const", bufs=1) as cp, \
             tc.tile_pool(name="drp", bufs=1, space="DRAM") as drp:

            # ---- constants into SBUF ----
            wint_sb = cp.tile([128, QD, h], f32)
            nc.sync.dma_start(wint_sb, wint_d.rearrange("(q p) h -> p q h", p=128))
            bin_sb = cp.tile([h, 1], f32)
            nc.sync.dma_start(bin_sb, bin_d)
            grow_sb = cp.tile([1, d], f32)
            nc.sync.dma_start(grow_sb, grow_d)
            brow_sb = cp.tile([1, d], f32)
            nc.sync.dma_start(brow_sb, brow_d)
            midwt_sb = cp.tile([h, n_layers, h], f32)
            nc.sync.dma_start(midwt_sb, midwt_d.rearrange("l k o -> k l o"))
            midg_sb = cp.tile([h, n_layers], f32)
            nc.sync.dma_start(midg_sb, midg_d)
            midbeta_sb = cp.tile([h, n_layers], f32)
            nc.sync.dma_start(midbeta_sb, midbeta_d)
            midbias_sb = cp.tile([h, n_layers], f32)
            nc.sync.dma_start(midbias_sb, midbias_d)
            woutt_sb = cp.tile([h, N_OUT], f32)
            nc.sync.dma_start(woutt_sb, woutt_d)
            bout_sb = cp.tile([1, 1], f32)
            nc.sync.dma_start(bout_sb, bout_d)
            identf = cp.tile([128, 128], f32)
            nc.sync.dma_start(identf, identf_d)
            identf2 = cp.tile([2, 2], f32)
            nc.sync.dma_start(identf2, identf2_d)
            onesbf = cp.tile([128, 1], bf16)
            nc.sync.dma_start(onesbf, onesbf_d)

            # fold-phase scratch pool (released before pass 2)
            sp = tc.alloc_tile_pool(name="fold", bufs=1)

            # =========== PASS 1: per-feature sum / sumsq of x ===========
            with tc.tile_pool(name="p1", bufs=6) as p1, \
                 tc.tile_pool(name="p1ps", bufs=1, space="PSUM") as p1ps:
                ps_sum = [p1ps.tile([1, 512], f32, name=f"ps_sum{c}") for c in range(CC)]
                ps_sq = [p1ps.tile([1, 512], f32, name=f"ps_sq{c}") for c in range(CC)]
                for i in range(NT):
                    xt = p1.tile([128, d], bf16, tag="xt", name=f"xt{i}")
                    nc.gpsimd.dma_start(xt, x_d[i * 128:(i + 1) * 128, :])  # f32->bf16 cast DMA
                    xsq = p1.tile([128, d], bf16, tag="xsq", name=f"xsq{i}", bufs=4)
                    nc.vector.tensor_tensor(out=xsq, in0=xt, in1=xt, op=ALU.mult)
                    for c in range(CC):
                        nc.tensor.matmul(ps_sum[c], onesbf, xt[:, c * 512:(c + 1) * 512],
                                         start=(i == 0), stop=(i == NT - 1),
                                         skip_group_check=True)
                    for c in range(CC):
                        nc.tensor.matmul(ps_sq[c], onesbf, xsq[:, c * 512:(c + 1) * 512],
                                         start=(i == 0), stop=(i == NT - 1),
                                         skip_group_check=True)

                stats_row = sp.tile([1, 2 * d], f32)
                for c in range(CC):
                    nc.vector.tensor_copy(stats_row[:, c * 512:(c + 1) * 512], ps_sum[c])
                for c in range(CC):
                    nc.vector.tensor_copy(stats_row[:, d + c * 512:d + (c + 1) * 512], ps_sq[c])

            # ---- AllReduce of [sum | sumsq] ----
            st1i = drp.tile([1, 2 * d], f32)
            st1o = drp.tile([1, 2 * d], f32)
            nc.sync.dma_start(st1i, stats_row)
            nc.gpsimd.collective_compute(
                "AllReduce", mybir.AluOpType.add, replica_groups=rg,
                ins=[st1i.opt()], outs=[st1o.opt()])
            g_row = stats_row  # reuse the SBUF row for the reduced result
            nc.sync.dma_start(g_row, st1o)

            # ---- stats -> (s, t) rows; Newton-polished rsqrt ----
            # register-style temp reuse to save SBUF columns
            mu = sp.tile([1, d], f32)
            tmp1 = sp.tile([1, d], f32)
            tmp2 = sp.tile([1, d], f32)
            tmp3 = sp.tile([1, d], f32)
            nc.vector.tensor_scalar_mul(mu, g_row[:, 0:d], 1.0 / B_TOT)
            vep = g_row[:, d:2 * d]
            nc.vector.tensor_scalar(out=vep, in0=vep,
                                    scalar1=1.0 / B_TOT, scalar2=float(EPS),
                                    op0=ALU.mult, op1=ALU.add)  # E[x^2]+eps
            nc.vector.tensor_tensor(out=tmp1, in0=mu, in1=mu, op=ALU.mult)  # mu^2
            nc.vector.tensor_tensor(out=vep, in0=vep, in1=tmp1,
                                    op=ALU.subtract)  # var+eps
            nc.scalar.activation(tmp2, vep, AF.Sqrt)         # sd
            nc.vector.reciprocal(tmp3, tmp2)                 # r0
            # one Newton step: r = r0*(1.5 - 0.5*vep*r0^2)
            nc.vector.tensor_tensor(out=tmp1, in0=tmp3, in1=tmp3, op=ALU.mult)  # r0^2
            nc.vector.tensor_tensor(out=tmp2, in0=vep, in1=tmp1, op=ALU.mult)
            nc.vector.tensor_scalar(out=tmp2, in0=tmp2, scalar1=-0.5, scalar2=1.5,
                                    op0=ALU.mult, op1=ALU.add)
            nc.vector.tensor_tensor(out=tmp1, in0=tmp3, in1=tmp2, op=ALU.mult)  # r
            s_row = tmp3
            nc.vector.tensor_tensor(out=s_row, in0=tmp1, in1=grow_sb, op=ALU.mult)
            nc.vector.tensor_tensor(out=tmp2, in0=mu, in1=s_row, op=ALU.mult)
            t_row = tmp1
            nc.vector.tensor_tensor(out=t_row, in0=brow_sb, in1=tmp2,
                                    op=ALU.subtract)  # t = beta - mu*s

            # transpose s,t rows into per-partition layout [128, QD]
            with tc.tile_pool(name="stps", bufs=1, space="PSUM") as stps:
                ps_st = stps.tile([128, QD, 2], f32)
                for q in range(QD):
                    nc.tensor.matmul(ps_st[:, q, 0:1],
                                     s_row[:, q * 128:(q + 1) * 128],
                                     identf2[0:1, 0:1], is_transpose=True,
                                     skip_group_check=True)
                    nc.tensor.matmul(ps_st[:, q, 1:2],
                                     t_row[:, q * 128:(q + 1) * 128],
                                     identf2[0:1, 0:1], is_transpose=True,
                                     skip_group_check=True)
                sT = sp.tile([128, QD], f32)
                tT = sp.tile([128, QD], f32)
                nc.vector.tensor_copy(sT, ps_st[:, :, 0])
                nc.vector.tensor_copy(tT, ps_st[:, :, 1])

            # fold: wfold[:,q,:] = wint[:,q,:] * sT[:,q], bias1 = b_in + W_in @ t
            wfold = cp.tile([128, QD, h], f32r)
            for q in range(QD):
                nc.vector.tensor_scalar_mul(wfold[:, q, :], wint_sb[:, q, :],
                                            sT[:, q:q + 1])
            with tc.tile_pool(name="pbias", bufs=1, space="PSUM") as pbias:
                ps_b1 = pbias.tile([h, 1], f32)
                for q in range(QD):
                    nc.tensor.matmul(ps_b1, wint_sb[:, q, :], tT[:, q:q + 1],
                                     start=(q == 0), stop=(q == QD - 1),
                                     skip_group_check=True)
                bias1 = cp.tile([h, 1], f32)
                nc.vector.tensor_tensor(out=bias1, in0=ps_b1, in1=bin_sb, op=ALU.add)

            sp.release()  # fold scratch freed before pass 2

            # h buffers, [feature, batch] layout, f32r
            hp = tc.alloc_tile_pool(name="hpool", bufs=1)
            h_a = hp.tile([h, b_local], f32r)
            h_b = hp.tile([h, b_local], f32r)

            # =========== PASS 2: h1 = relu(x_norm @ W_in'.T + bias1) ===========
            with tc.tile_pool(name="p2x", bufs=6) as p2x, \
                 tc.tile_pool(name="p2t", bufs=24 if QD == 16 else 3 * QD) as p2t, \
                 tc.tile_pool(name="p2ps", bufs=3, space="PSUM") as p2ps, \
                 tc.tile_pool(name="p2ph", bufs=2, space="PSUM") as p2ph:
                for bc in range(BCN):
                    xbs = []
                    for t4 in range(4):
                        xb = p2x.tile([128, d], f32, tag="x2", name=f"x2_{bc}_{t4}")
                        r0_ = bc * 512 + t4 * 128
                        nc.sync.dma_start(xb, x_d[r0_:r0_ + 128, :])
                        xbs.append(xb)
                    xts = []
                    for q in range(QD):
                        pst = p2ps.tile([128, 512], f32, tag="pst", name=f"pst{bc}_{q}")
                        for t4 in range(4):
                            nc.tensor.matmul(pst[:, t4 * 128:(t4 + 1) * 128],
                                             xbs[t4][:, q * 128:(q + 1) * 128],
                                             identf, is_transpose=True,
                                             skip_group_check=True)
                        xtq = p2t.tile([128, 512], f32r, tag="xT", name=f"xT{bc}_{q}")
                        if q % 2 == 0:
                            nc.vector.tensor_copy(xtq, pst)
                        else:
                            nc.scalar.copy(xtq, pst)
                        xts.append(xtq)
                    psh = p2ph.tile([h, 512], f32, tag="psh", name=f"psh{bc}")
                    for q in range(QD):
                        nc.tensor.matmul(psh, wfold[:, q, :], xts[q],
                                         start=(q == 0), stop=(q == QD - 1),
                                         skip_group_check=True)
                    nc.scalar.activation(h_a[:, bc * 512:(bc + 1) * 512], psh,
                                         AF.Relu, bias=bias1[:, 0:1])

            # =========== 12 middle layers ===========
            h_in, h_out = h_a, h_b
            with tc.tile_pool(name="mid", bufs=2) as mp_, \
                 tc.tile_pool(name="midps", bufs=2, space="PSUM") as mps, \
                 tc.tile_pool(name="midpb", bufs=1, space="PSUM") as mpb:
                for l in range(n_layers):
                    bnst = mp_.tile([h, BCN, 6], f32, tag="bnst", name=f"bnst{l}")
                    h_in_f = h_in.bitcast(f32)
                    for bc in range(BCN):
                        nc.vector.bn_stats(bnst[:, bc, :],
                                           h_in_f[:, bc * 512:(bc + 1) * 512])
                    mv = mp_.tile([h, 2], f32, tag="mv", name=f"mv{l}")
                    nc.vector.bn_aggr(mv, bnst)
                    # payload [mean, var+mean^2] = [mean, E[h^2]]
                    pay = mp_.tile([h, 2], f32, tag="pay", name=f"pay{l}")
                    nc.vector.tensor_copy(pay[:, 0:1], mv[:, 0:1])
                    msq = mp_.tile([h, 1], f32, tag="msq", name=f"msq{l}")
                    nc.vector.tensor_tensor(out=msq, in0=mv[:, 0:1], in1=mv[:, 0:1],
                                            op=ALU.mult)
                    nc.vector.tensor_tensor(out=pay[:, 1:2], in0=mv[:, 1:2], in1=msq,
                                            op=ALU.add)
                    mbi = drp.tile([h, 2], f32, name=f"mbi{l}")
                    mbo = drp.tile([h, 2], f32, name=f"mbo{l}")
                    nc.sync.dma_start(mbi, pay)
                    nc.gpsimd.collective_compute(
                        "AllReduce", mybir.AluOpType.add, replica_groups=rg,
                        ins=[mbi.opt()], outs=[mbo.opt()])
                    g2 = mp_.tile([h, 2], f32, tag="g2", name=f"g2{l}")
                    nc.sync.dma_start(g2, mbo)

                    mug = mp_.tile([h, 1], f32, tag="mug", name=f"mug{l}")
                    nc.vector.tensor_scalar_mul(mug, g2[:, 0:1], 1.0 / n_cores)
                    veg = mp_.tile([h, 1], f32, tag="veg", name=f"veg{l}")
                    nc.vector.tensor_scalar(out=veg, in0=g2[:, 1:2],
                                            scalar1=1.0 / n_cores, scalar2=float(EPS),
                                            op0=ALU.mult, op1=ALU.add)
                    musq2 = mp_.tile([h, 1], f32, tag="musq2", name=f"musq2{l}")
                    nc.vector.tensor_tensor(out=musq2, in0=mug, in1=mug, op=ALU.mult)
                    vef = mp_.tile([h, 1], f32, tag="vef", name=f"vef{l}")
                    nc.vector.tensor_tensor(out=vef, in0=veg, in1=musq2, op=ALU.subtract)
                    sd2 = mp_.tile([h, 1], f32, tag="sd2", name=f"sd2{l}")
                    nc.scalar.activation(sd2, vef, AF.Sqrt)
                    rr = mp_.tile([h, 1], f32, tag="rr", name=f"rr{l}")
                    nc.vector.reciprocal(rr, sd2)
                    s2 = mp_.tile([h, 1], f32, tag="s2", name=f"s2{l}")
                    nc.vector.tensor_tensor(out=s2, in0=rr, in1=midg_sb[:, l:l + 1],
                                            op=ALU.mult)
                    mt = mp_.tile([h, 1], f32, tag="mt", name=f"mt{l}")
                    nc.vector.tensor_tensor(out=mt, in0=mug, in1=s2, op=ALU.mult)
                    t2 = mp_.tile([h, 1], f32, tag="t2", name=f"t2{l}")
                    nc.vector.tensor_tensor(out=t2, in0=midbeta_sb[:, l:l + 1], in1=mt,
                                            op=ALU.subtract)
                    wf = mp_.tile([h, h], f32r, tag="wf", name=f"wf{l}")
                    nc.vector.tensor_scalar_mul(wf, midwt_sb[:, l, :], s2)
                    ps_b2 = mpb.tile([h, 1], f32, tag="psb2", name=f"psb2_{l}")
                    nc.tensor.matmul(ps_b2, midwt_sb[:, l, :], t2,
                                     skip_group_check=True)
                    bias2 = mp_.tile([h, 1], f32, tag="bias2", name=f"bias2{l}")
                    nc.vector.tensor_tensor(out=bias2, in0=ps_b2,
                                            in1=midbias_sb[:, l:l + 1], op=ALU.add)
                    for bc in range(BCN):
                        psm = mps.tile([h, 512], f32, tag="psm", name=f"psm{l}_{bc}")
                        nc.tensor.matmul(psm, wf, h_in[:, bc * 512:(bc + 1) * 512],
                                         skip_group_check=True)
                        nc.scalar.activation(h_out[:, bc * 512:(bc + 1) * 512], psm,
                                             AF.Relu, bias=bias2[:, 0:1])
                    h_in, h_out = h_out, h_in

                # =========== head: out = h @ W_out.T + b_out ===========
                woutt_r = mp_.tile([h, N_OUT], f32r, bufs=1)
                nc.vector.tensor_copy(woutt_r, woutt_sb)
                out_row = mp_.tile([1, b_local], f32, bufs=1)
                for bc in range(BCN):
                    pso = mps.tile([1, 512], f32, tag="pso", name=f"pso{bc}")
                    nc.tensor.matmul(pso, woutt_r, h_in[:, bc * 512:(bc + 1) * 512],
                                     skip_group_check=True)
                    nc.scalar.activation(out_row[:, bc * 512:(bc + 1) * 512], pso,
                                         AF.Identity, bias=bout_sb[0:1, 0:1])
                nc.sync.dma_start(out_d.rearrange("b o -> o b"), out_row)
            hp.release()

    nc.compile()
    return nc


def make_in_maps(inputs, n_cores=N_CORES, b_local=B // N_CORES):
    """Host-side preprocessing: shard x, pre-transpose weights, replicate."""
    x = np.asarray(inputs["x"], np.float32)
    wint = np.ascontiguousarray(np.asarray(inputs["W_in"], np.float32).T)
    bin_ = np.asarray(inputs["b_in"], np.float32).reshape(-1, 1)
    grow = np.asarray(inputs["bn_gamma_in"], np.float32).reshape(1, -1)
    brow = np.asarray(inputs["bn_beta_in"], np.float32).reshape(1, -1)
    midwt = np.ascontiguousarray(
        np.asarray(inputs["mid_W"], np.float32).transpose(0, 2, 1))
    midg = np.ascontiguousarray(np.asarray(inputs["mid_gamma"], np.float32).T)
    midbeta = np.ascontiguousarray(np.asarray(inputs["mid_beta"], np.float32).T)
    midbias = np.ascontiguousarray(np.asarray(inputs["mid_b"], np.float32).T)
    woutt = np.ascontiguousarray(np.asarray(inputs["W_out"], np.float32).T)
    bout = np.asarray(inputs["b_out"], np.float32).reshape(1, 1)
    identf = np.eye(128, dtype=np.float32)
    identf2 = np.eye(2, dtype=np.float32)
    onesbf = np.ones((128, 1), dtype=BF16)

    common = dict(wint=wint, bin=bin_, grow=grow, brow=brow, midwt=midwt,
                  midg=midg, midbeta=midbeta, midbias=midbias, woutt=woutt,
                  bout=bout, identf=identf, identf2=identf2, onesbf=onesbf)
    in_maps = []
    for c in range(n_cores):
        m = dict(common)
        m["x"] = np.ascontiguousarray(x[c * b_local:(c + 1) * b_local])
        in_maps.append(m)
    return in_maps


@functools.lru_cache(maxsize=1)
def _get_program():
    return build_program()


def kernel(**inputs) -> np.ndarray:
    from concourse.bass_utils import run_bass_kernel_spmd
    nc = _get_program()
    in_maps = make_in_maps(inputs)
    res = run_bass_kernel_spmd(nc, in_maps, core_ids=list(range(N_CORES)))
    out = np.concatenate([res.results[c]["out"] for c in range(N_CORES)], axis=0)
    return out.astype(np.float32)


if __name__ == "__main__":
    # smoke-build
    nc = build_program(n_cores=2, b_local=1024, d=512, n_layers=2)
    print("built ok:", len(nc.inst_map), "instructions")
